# revision 1
# baseline (speedup 1.0000x reference)
"""Trainium2 Bass kernel for nn_DilatedSpatioTemporalGCN.

Key observation: the reference masks adjacency with (softmax(...) > 0), and a
softmax output is strictly positive for these input magnitudes, so both
normalized adjacencies collapse to the constant matrix (J + I) / 513. The
static_MTE_matrix and dynamic-score einsum therefore do not affect the output
at all and are never read. What remains per batch element:

  per layer l (M_l = Ws[l] @ Wg[:F] + Wd[l] @ Wg[F:], folded through the gate):
    Z = c2 * (X + 1 colsum(X)^T) @ M_l + bias_row   (c2 = 1/513 with fp32 rounding)
    g = sigmoid(Z)
    r_l = relu(causal dilated conv_t(g) + bc[l]);  X_{l+1} = X + sum_j r_j
  then a tiny attention over the three r_l[t=T-1] slices.

Layout on-chip: partition p = (3 - t mod 4) * 32 + f, free = (t div 4, n);
the reversed group order puts t = T-1 at partition base 0 so the attention
matmuls can read it directly. All feature transforms become 128x128
block-structured fp32r matmuls at full PE width; time shifts in the conv
become free-dim slot offsets plus partition-block shifts encoded in the
weight matrices. Taps that would read the causal zero padding are skipped.

Sharding: data-parallel over batch (4 elements) on cores 0-3; cores 4-7
run duplicate batches (harmless, keeps all 8 cores uniform).
"""

import os
import numpy as np
from contextlib import ExitStack

import concourse.bacc as bacc
import concourse.tile as tile
from concourse import mybir
from concourse.bass_utils import run_bass_kernel_spmd

F32 = mybir.dt.float32
F32R = mybir.dt.float32r
AF = mybir.ActivationFunctionType
ALU = mybir.AluOpType

BSZ, T, FD, N = 4, 12, 32, 512
L, K = 3, 3
DIL = (1, 2, 4)
NTHI = 3          # t div 4 chunks
NCHUNK = 4        # n chunks of 128

# logical time t lives at partition group 3 - (t % 4), free chunk t // 4.
# Host sends timesteps permuted so the natural DMA layout lands like that.
T_PERM = [4 * (t // 4) + 3 - (t % 4) for t in range(T)]

# wr column layout
ZW_OFF = 0                      # 3 x 128
CW_OFF = ZW_OFF + 3 * 128       # 12 x 128 (conv lhsT list, variable per layer)
CW_COUNTS = (5, 4, 3)
WCR_OFF = CW_OFF + 12 * 128     # 9 x 32 (wc rhs for res_T mms, rows 0:32)
WA_OFF = WCR_OFF + 9 * 32       # 32     (Wa, rows 0:32)
V_OFF = WA_OFF + 32             # 2      (v padded to M=2, rows 0:32)
WR_COLS = V_OFF + 2
WR_SPLIT = CW_OFF + CW_COUNTS[0] * 128   # first slice: Zw + layer-0 conv weights

# wf column layout: 0-2 bias_row per layer, 3-5 bc per layer, 6 ba, 7 zeros
WF_COLS = 8

_CACHE = {}


def _conv_plan(l):
    """Per layer: list of (k, carry, {q_out: q_in}) matmul groups in the
    reversed-group layout."""
    d = DIL[l]
    plan = []
    for k in range(K):
        delta = (K - 1 - k) * d
        groups = {}
        for q_out in range(4):
            a = 3 - q_out                  # tau % 4
            ap_ = (a - delta) % 4
            carry = (a - delta - ap_) // 4
            groups.setdefault(carry, {})[q_out] = 3 - ap_
        for carry in sorted(groups):
            plan.append((k, carry, groups[carry]))
    return plan


def _rt_slices(l):
    """(group, slot) of g feeding the t=T-1 conv tap, per k."""
    d = DIL[l]
    out = []
    for k in range(K):
        tp = (T - 1) - (K - 1 - k) * d
        out.append((3 - (tp % 4), tp // 4))
    return out


def _build_nc():
    nc = bacc.Bacc("TRN2", target_bir_lowering=False, debug=False)
    emb_d = nc.dram_tensor("emb", [T, FD, N], F32R, kind="ExternalInput").ap()
    wr_d = nc.dram_tensor("wr", [128, WR_COLS], F32R, kind="ExternalInput").ap()
    wf_d = nc.dram_tensor("wf", [128, WF_COLS], F32, kind="ExternalInput").ap()
    y_d = nc.dram_tensor("y", [N, FD], F32, kind="ExternalOutput").ap()

    trace_sim = bool(int(os.environ.get("K_TRACE_SIM", "0")))
    with tile.TileContext(nc, trace_sim=trace_sim) as tc, ExitStack() as ctx:
        ctx.enter_context(nc.allow_low_precision(
            "fp32r tiles feed fp32r matmuls; accumulation stays fp32 in PSUM"))
        wpool = ctx.enter_context(tc.tile_pool(name="w", bufs=1))
        spool = ctx.enter_context(tc.tile_pool(name="s", bufs=1))
        ppool_z = ctx.enter_context(tc.tile_pool(name="pz", bufs=3, space="PSUM"))
        ppool_c = ctx.enter_context(tc.tile_pool(name="pc", bufs=2, space="PSUM"))
        ppool_s = ctx.enter_context(tc.tile_pool(name="psm", bufs=2, space="PSUM"))

        # ---- loads: emb on the SP queue, weights on the (idle) GPSIMD queue ----
        embs = [wpool.tile([128, N], F32R, name=f"emb{c}") for c in range(NTHI)]
        emb_view = emb_d.rearrange("(thi tlo) f n -> (tlo f) thi n", tlo=4)
        nc.sync.dma_start(out=embs[0][:], in_=emb_view[:, 0, :])
        nc.scalar.dma_start(out=embs[1][:], in_=emb_view[:, 1, :])
        nc.scalar.dma_start(out=embs[2][:], in_=emb_view[:, 2, :])

        wr = wpool.tile([128, WR_COLS], F32R)
        wf = wpool.tile([128, WF_COLS], F32)
        nc.gpsimd.dma_start(out=wr[:, 0:CW_OFF], in_=wr_d[:, 0:CW_OFF])
        nc.gpsimd.dma_start(out=wr[:, CW_OFF:WR_SPLIT],
                            in_=wr_d[:, CW_OFF:WR_SPLIT])
        nc.gpsimd.dma_start(out=wf, in_=wf_d)
        nc.gpsimd.dma_start(out=wr[:, WR_SPLIT:], in_=wr_d[:, WR_SPLIT:])

        # ---- ACT table warmup off the critical path: the sigmoid set also
        # serves Relu and Tanh for the layer phase ----
        dumin = spool.tile([128, 2], F32)
        nc.vector.memset(dumin[:], 0.0)
        duout = spool.tile([128, 2], F32)
        nc.scalar.activation(duout[:], dumin[:], AF.Sigmoid)
        # PE p-state warmup on zeros
        dumm = spool.tile([128, 4], F32R)
        nc.vector.memset(dumm[:].bitcast(F32), 0.0)
        pwarm = ppool_s.tile([2, 2], F32, tag="small", name="pwarm", bufs=1)
        nc.tensor.matmul(pwarm[:], dumm[:, 0:2], dumm[:, 2:4])

        # colsum state sx_l and relu accumulators (layer 2's is never used)
        sx = [spool.tile([128, 4], F32R, tag=f"sx{l}", name=f"sx{l}")
              for l in range(L)]
        racc = [spool.tile([128, 4], F32, tag=f"racc{l}", name=f"racc{l}")
                for l in range(L - 1)]
        for t_ in racc:
            nc.vector.memset(t_[:], 0.0)
        nc.vector.memset(sx[0][:, 1:4].bitcast(F32), 0.0)
        # per-chunk column sums of the initial embedding
        nc.vector.reduce_sum(sx[0][:, 0:1], embs[0][:].bitcast(F32),
                             axis=mybir.AxisListType.X)
        nc.vector.reduce_sum(sx[0][:, 1:2], embs[1][:].bitcast(F32),
                             axis=mybir.AxisListType.X)

        gps = [[wpool.tile([128, N], F32R, name=f"g{l}{c}") for c in range(NTHI)]
               for l in range(L)]
        rs = [[wpool.tile([128, N], F32R, name=f"r{l}{c}") for c in range(NTHI)]
              for l in range(L)]
        rt_sb = wpool.tile([128, NCHUNK, L, FD], F32)
        sv_flat = wpool.tile([1, L * N], F32)
        nstage = [sum(1 for q, _ in _rt_slices(l) if q != 0) for l in range(L)]
        rstages = [spool.tile([32, max(nstage[l], 1), N], F32R, name=f"rstage{l}")
                   for l in range(L)]
        prts = [ppool_s.tile([128, NCHUNK, FD], F32, tag="prt", name=f"prt{l}",
                             bufs=2)
                for l in range(L)]
        s_sbs = [spool.tile([32, N], F32R, name=f"s_sb{l}") for l in range(L)]
        svg = spool.tile([128, NCHUNK, 3], F32)
        e_sb = spool.tile([128, NCHUNK, 3], F32)
        es = spool.tile([128, NCHUNK], F32)
        ri = spool.tile([128, NCHUNK], F32)
        y_sb = spool.tile([128, NCHUNK, FD], F32)
        tmps = [spool.tile([128, FD], F32, name=f"ytmp{i}") for i in range(4)]

        def softmax_chunk(c):
            """Regroup DMA; runs as soon as sv_flat bytes for chunk c exist."""
            deng = nc.sync if c % 2 == 0 else nc.gpsimd
            deng.dma_start(
                out=svg[:, c, :],
                in_=sv_flat[0:1, 384 * c: 384 * (c + 1)].rearrange(
                    "p (m r) -> p m r", r=3))

        def attn_logits_a(l):
            """s = tanh(Wa^T r_l[T-1] + ba); r_l[T-1] is at base 0 here."""
            ps = ppool_z.tile([32, N], F32, tag="zb", name=f"ps{l}")
            nc.tensor.matmul(ps[:], wr[0:32, WA_OFF:WA_OFF + 32],
                             rs[l][NTHI - 1][0:32, :])
            nc.scalar.activation(s_sbs[l][:], ps[:], AF.Tanh,
                                 bias=wf[0:32, 6:7], scale=1.0)

        def attn_logits_b(l):
            psv = ppool_c.tile([2, N], F32, tag="cv", name=f"psv{l}")
            nc.tensor.matmul(psv[:], wr[0:32, V_OFF:V_OFF + 2], s_sbs[l][:])
            nc.vector.tensor_copy(sv_flat[0:1, l * N:(l + 1) * N], psv[0:1, :])
            if l == 0:
                softmax_chunk(0)
            elif l == 1:
                softmax_chunk(1)
            else:
                softmax_chunk(2)
                softmax_chunk(3)

        def mix_chunk(c):
            """Final weighted mix; needs rt_sb of all layers."""
            eng = nc.vector if c % 2 == 0 else nc.gpsimd
            ta, tb = tmps[c], tmps[(c + 2) % 4]
            if c % 2 == 0:
                eng.tensor_scalar(ta[:], rt_sb[:, c, 0, :], e_sb[:, c, 0:1],
                                  None, ALU.mult)
                eng.scalar_tensor_tensor(ta[:], rt_sb[:, c, 1, :],
                                         e_sb[:, c, 1:2], ta[:],
                                         ALU.mult, ALU.add)
                eng.scalar_tensor_tensor(ta[:], rt_sb[:, c, 2, :],
                                         e_sb[:, c, 2:3], ta[:],
                                         ALU.mult, ALU.add)
            else:
                eng.tensor_scalar(ta[:], rt_sb[:, c, 0, :], e_sb[:, c, 0:1],
                                  None, ALU.mult)
                eng.tensor_scalar(tb[:], rt_sb[:, c, 1, :], e_sb[:, c, 1:2],
                                  None, ALU.mult)
                eng.tensor_tensor(ta[:], ta[:], tb[:], ALU.add)
                eng.tensor_scalar(tb[:], rt_sb[:, c, 2, :], e_sb[:, c, 2:3],
                                  None, ALU.mult)
                eng.tensor_tensor(ta[:], ta[:], tb[:], ALU.add)
            eng.tensor_scalar(y_sb[:, c, :], ta[:], ri[:, c:c + 1], None,
                              ALU.mult)

        for l in range(L):
            zw = wr[:, ZW_OFF + l * 128: ZW_OFF + (l + 1) * 128]
            cw_base = CW_OFF + sum(CW_COUNTS[:l]) * 128
            plan = _conv_plan(l)
            gp = gps[l]
            last = l == L - 1

            # bias vec for the sigmoid: Sz = blockdiag(Mc)^T @ sx_l, split in
            # two column groups so chunk-0 work isn't gated on the last chunk
            psz = ppool_s.tile([128, 8], F32, tag="small", name=f"psz{l}", bufs=1)
            bv = spool.tile([128, 4], F32, tag=f"bv{l}", name=f"bv{l}")
            if l == 0:
                # three stages so sigmoid chunk c waits only on reduce c
                nc.tensor.matmul(psz[:, 0:2], zw, sx[l][:, 0:2])
                nc.vector.tensor_scalar(bv[:, 0:1], psz[:, 0:1],
                                        wf[:, l:l + 1], None, ALU.add)
                nc.tensor.matmul(psz[:, 2:4], zw, sx[l][:, 0:2])
                nc.vector.tensor_scalar(bv[:, 1:2], psz[:, 3:4],
                                        wf[:, l:l + 1], None, ALU.add)
                nc.vector.reduce_sum(sx[0][:, 2:3], embs[2][:].bitcast(F32),
                                     axis=mybir.AxisListType.X)
                nc.tensor.matmul(psz[:, 4:6], zw, sx[l][:, 2:4])
                nc.vector.tensor_scalar(bv[:, 2:3], psz[:, 4:5],
                                        wf[:, l:l + 1], None, ALU.add)
            else:
                nc.tensor.matmul(psz[:, 0:2], zw, sx[l][:, 0:2])
                nc.vector.tensor_scalar(bv[:, 0:2], psz[:, 0:2],
                                        wf[:, l:l + 1], None, ALU.add)
                nc.tensor.matmul(psz[:, 2:4], zw, sx[l][:, 2:4])
                nc.vector.tensor_scalar(bv[:, 2:3], psz[:, 2:3],
                                        wf[:, l:l + 1], None, ALU.add)

            # Z matmuls + sigmoids (chunk-major so ACT never waits on conv)
            pzs = []
            for c in range(NTHI):
                pz = ppool_z.tile([128, N], F32, tag="zb", name=f"pz{l}{c}")
                rhss = [embs[c][:]] + [rs[j][c][:] for j in range(l)]
                for i, rhs in enumerate(rhss):
                    nc.tensor.matmul(pz[:], zw, rhs,
                                     start=(i == 0), stop=(i == len(rhss) - 1))
                pzs.append(pz)
            for c in range(NTHI):
                nc.scalar.activation(gp[c][:], pzs[c][:], AF.Sigmoid,
                                     bias=bv[:, c:c + 1], scale=1.0)
            if l > 0:
                attn_logits_a(l - 1)

            # conv matmuls + relu(+colsum accum). The final layer only needs
            # chunk 2 (its other outputs feed nothing).
            for c in ((NTHI - 1,) if last else range(NTHI)):
                mms = [(i, k, carry) for i, (k, carry, _) in enumerate(plan)
                       if c + carry >= 0]
                pc = ppool_c.tile([128, N], F32, tag="cv", name=f"pc{l}{c}")
                for j, (i, k, carry) in enumerate(mms):
                    nc.tensor.matmul(
                        pc[:], wr[:, cw_base + i * 128: cw_base + (i + 1) * 128],
                        gp[c + carry][:],
                        start=(j == 0), stop=(j == len(mms) - 1))
                if last:
                    nc.scalar.activation(rs[l][c][:], pc[:], AF.Relu,
                                         bias=wf[:, 3 + l:4 + l], scale=1.0)
                else:
                    nc.scalar.activation(rs[l][c][:], pc[:], AF.Relu,
                                         bias=wf[:, 3 + l:4 + l], scale=1.0,
                                         accum_out=racc[l][:, c:c + 1])
                    if c == 1:
                        nc.vector.tensor_add(sx[l + 1][:, 0:2],
                                             sx[l][:, 0:2].bitcast(F32),
                                             racc[l][:, 0:2])
                    elif c == 2:
                        nc.vector.tensor_add(sx[l + 1][:, 2:4],
                                             sx[l][:, 2:4].bitcast(F32),
                                             racc[l][:, 2:4])

            if l > 0:
                attn_logits_b(l - 1)

            # stage the g slices feeding res_agg[l] whose group isn't 0
            slices_ = _rt_slices(l)
            lhs_list = []
            si = 0
            for k, (q, slot) in enumerate(slices_):
                if q == 0:
                    lhs_list.append(gp[slot])
                else:
                    eng = nc.sync if si % 2 == 0 else nc.gpsimd
                    eng.dma_start(out=rstages[l][:, si, :],
                                  in_=gp[slot][32 * q: 32 * (q + 1), :])
                    lhs_list.append((rstages[l], si))
                    si += 1

            # res_agg[l] transposed ([n, fo]) + relu on DVE
            for c in range(NCHUNK):
                for k in range(K):
                    src = lhs_list[k]
                    lhsT = (src[0][:, src[1], 128 * c: 128 * (c + 1)]
                            if isinstance(src, tuple)
                            else src[0:32, 128 * c: 128 * (c + 1)])
                    nc.tensor.matmul(
                        prts[l][:, c, :], lhsT,
                        wr[0:32, WCR_OFF + (l * K + k) * 32:
                                 WCR_OFF + (l * K + k + 1) * 32],
                        start=(k == 0), stop=(k == K - 1))
            nc.vector.tensor_scalar(rt_sb[:, :, l, :], prts[l][:],
                                    wf[:, 3 + l:4 + l], 0.0, ALU.add, ALU.max)
            if last:
                attn_logits_a(l)
                # load the exp table in the ACT gap while the regroup DMAs fly;
                # the input aliases tanh output so this cannot be hoisted
                nc.scalar.activation(duout[0:32, :], s_sbs[l][:, 0:2].bitcast(F32),
                                     AF.Exp)
                attn_logits_b(l)

        nc.scalar.activation(e_sb[:, 0:2, :], svg[:, 0:2, :], AF.Exp)
        nc.vector.tensor_reduce(es[:, 0:2], e_sb[:, 0:2, :],
                                axis=mybir.AxisListType.X, op=ALU.add)
        nc.vector.reciprocal(ri[:, 0:2], es[:, 0:2])
        mix_chunk(0)
        mix_chunk(1)
        nc.scalar.activation(e_sb[:, 2:4, :], svg[:, 2:4, :], AF.Exp)
        nc.vector.tensor_reduce(es[:, 2:4], e_sb[:, 2:4, :],
                                axis=mybir.AxisListType.X, op=ALU.add)
        nc.vector.reciprocal(ri[:, 2:4], es[:, 2:4])
        mix_chunk(2)
        mix_chunk(3)
        nc.sync.dma_start(out=y_d.rearrange("(c p) f -> p c f", p=128), in_=y_sb)

    nc.finalize()
    return nc


def _host_weights(Wd, bd, Ws, bs, Wg, bg, Wc, bc, Wa, ba, v):
    f32 = np.float32
    dinv = f32(1.0) / np.sqrt(f32(513.0))
    c2 = f32(dinv * dinv)

    wr = np.zeros((128, WR_COLS), np.float32)
    wf = np.zeros((128, WF_COLS), np.float32)

    for l in range(L):
        M = (Ws[l] @ Wg[:FD] + Wd[l] @ Wg[FD:]).astype(f32)
        Mc = (c2 * M).astype(f32)
        for q in range(4):
            wr[32 * q:32 * (q + 1),
               ZW_OFF + l * 128 + 32 * q: ZW_OFF + l * 128 + 32 * (q + 1)] = Mc

        cw_base = CW_OFF + sum(CW_COUNTS[:l]) * 128
        plan = _conv_plan(l)
        assert len(plan) == CW_COUNTS[l]
        for i, (k, carry, groups) in enumerate(plan):
            blk = Wc[l][:, :, 0, k].T.astype(f32)   # [fi, fo]
            for q_out, q_in in groups.items():
                wr[32 * q_in:32 * (q_in + 1),
                   cw_base + i * 128 + 32 * q_out:
                   cw_base + i * 128 + 32 * (q_out + 1)] = blk

        for k in range(K):
            wr[0:32, WCR_OFF + (l * K + k) * 32:
                     WCR_OFF + (l * K + k + 1) * 32] = Wc[l][:, :, 0, k].T

        bias_row = (bs[l] @ Wg[:FD] + bd[l] @ Wg[FD:] + bg).astype(f32)
        wf[:, l] = np.tile(bias_row, 4)
        wf[:, 3 + l] = np.tile(bc[l].astype(f32), 4)

    wr[0:32, WA_OFF:WA_OFF + 32] = Wa.astype(f32)
    wr[0:32, V_OFF:V_OFF + 1] = v.astype(f32)
    wf[0:32, 6] = ba.astype(f32)
    return wr, wf


def kernel(**inputs):
    node_embeddings = np.asarray(inputs["node_embeddings"], dtype=np.float32)
    wr, wf = _host_weights(
        np.asarray(inputs["Wd"], np.float32), np.asarray(inputs["bd"], np.float32),
        np.asarray(inputs["Ws"], np.float32), np.asarray(inputs["bs"], np.float32),
        np.asarray(inputs["Wg"], np.float32), np.asarray(inputs["bg"], np.float32),
        np.asarray(inputs["Wc"], np.float32), np.asarray(inputs["bc"], np.float32),
        np.asarray(inputs["Wa"], np.float32), np.asarray(inputs["ba"], np.float32),
        np.asarray(inputs["v"], np.float32),
    )

    if "nc" not in _CACHE:
        _CACHE["nc"] = _build_nc()
    nc = _CACHE["nc"]

    n_cores = 8
    in_maps = [
        {"emb": np.ascontiguousarray(node_embeddings[i % BSZ][T_PERM]),
         "wr": wr, "wf": wf}
        for i in range(n_cores)
    ]
    res = run_bass_kernel_spmd(nc, in_maps, core_ids=list(range(n_cores)))
    y = np.stack([res.results[b]["y"] for b in range(BSZ)], axis=0)
    return y.astype(np.float32)

